# revision 7
# baseline (speedup 1.0000x reference)
"""3x3 median filter (reflect padding) on Trainium2, data-parallel over 8 cores.

Problem: img [8, 3, 1024, 1024] f32 -> median-of-9 per pixel, same shape.
Sharding: batch dim across the 8 NeuronCores (one image per core, SPMD).

Per-core algorithm (image [3, 1024, 1024]):

  Pre-pass: build a reflect-padded copy of each channel in DRAM
  (pimg [C, H+2, PITCH]) by streaming rows through SBUF (the width
  padding costs two 1-column copies per row tile; the top/bottom
  padded rows are extra 1-row DMA writes).

  Main pass: every SBUF partition processes an independent sub-block
  of R=32 output rows x Cs=64 output cols, loading (R+2)x(Cs+2) with
  a 1-pixel halo from the padded image (one big overlapping-window
  DMA per 128-block iteration). Both window directions are then
  free-dimension shifts, so the whole median network runs as plain
  tensor_tensor min/max ops:

    stage H (col shifts): sort each horizontal triple -> rmin/rmed/rmax
    stage V (row shifts) + final:
        med9 = med3( max3_rows(rmin), med3_rows(rmed), min3_rows(rmax) )
    with med3(a,b,c) = max(min(a,b), min(max(a,b), c)).

  18 min/max tensor_tensor ops per pixel total, split between the
  Vector (DVE) and GPSIMD engines.
"""

import numpy as np

import concourse.bacc as bacc
import concourse.bass as bass
import concourse.mybir as mybir
from concourse.tile import TileContext

F32 = mybir.dt.float32
MIN = mybir.AluOpType.min
MAX = mybir.AluOpType.max

B, C, H, W = 8, 3, 1024, 1024
N_CORES = 8

# Default GPSIMD engine assignment (subset of the 18 network ops).
GPS_DEFAULT = frozenset()


def mk_ap(handle, offset, dims):
    """Manual access pattern: dims = [[step, count], ...] in elements."""
    return bass.AP(handle, offset, [list(d) for d in dims])


def build_nc(c=C, h=H, w=W, r=32, cs=64, gpsimd_ops=GPS_DEFAULT):
    assert h % r == 0 and w % cs == 0
    rb_total = h // r          # row blocks per channel
    s_total = w // cs          # col strips per channel
    rb_per_iter = max(1, 128 // s_total)
    pitch = ((w + 2) + 7) // 8 * 8   # padded-image row pitch (f32 elems)
    ph = h + 2

    nc = bacc.Bacc(None, target_bir_lowering=False, debug=False)
    img = nc.dram_tensor("img", [c, h, w], F32, kind="ExternalInput")
    out = nc.dram_tensor("out", [c, h, w], F32, kind="ExternalOutput")
    pimg = nc.dram_tensor("pimg", [c, ph, pitch], F32)

    def eng(name):
        return nc.gpsimd if name in gpsimd_ops else nc.vector

    with TileContext(nc) as tc:
        with (
            tc.tile_pool(name="pre", bufs=3) as pre,
            tc.tile_pool(name="io", bufs=2) as io,
            tc.tile_pool(name="wk", bufs=2) as wk,
        ):
            # ---------------- pre-pass: reflect-padded image in DRAM --------
            for ch in range(c):
                r0 = 0
                while r0 < h:
                    n = min(128, h - r0)
                    rt = pre.tile([128, w + 2], F32, tag="rt")
                    nc.sync.dma_start(out=rt[0:n, 1 : w + 1], in_=img[ch, r0 : r0 + n, :])
                    nc.scalar.copy(out=rt[0:n, 0:1], in_=rt[0:n, 2:3])
                    nc.scalar.copy(out=rt[0:n, w + 1 : w + 2], in_=rt[0:n, w - 1 : w])
                    nc.sync.dma_start(
                        out=mk_ap(pimg, ch * ph * pitch + (1 + r0) * pitch,
                                  [[pitch, n], [1, w + 2]]),
                        in_=rt[0:n, :],
                    )
                    if r0 == 0:  # padded top row = img row 1 (reflect)
                        nc.sync.dma_start(
                            out=mk_ap(pimg, ch * ph * pitch, [[pitch, 1], [1, w + 2]]),
                            in_=rt[1:2, :],
                        )
                    if r0 + n == h:  # padded bottom row = img row h-2
                        pbot = h - 2 - r0
                        nc.sync.dma_start(
                            out=mk_ap(pimg, ch * ph * pitch + (ph - 1) * pitch,
                                      [[pitch, 1], [1, w + 2]]),
                            in_=rt[pbot : pbot + 1, :],
                        )
                    r0 += n

            # ---------------- main pass: block-strip median -----------------
            for ch in range(c):
                rb0 = 0
                while rb0 < rb_total:
                    nrb = min(rb_per_iter, rb_total - rb0)
                    nj = nrb * s_total  # jobs (partitions) this iteration
                    t = io.tile([128, r + 2, cs + 2], F32, tag="t")
                    for k in range(nrb):
                        nc.sync.dma_start(
                            out=t[k * s_total : (k + 1) * s_total],
                            in_=mk_ap(
                                pimg,
                                ch * ph * pitch + (rb0 + k) * r * pitch,
                                [[cs, s_total], [pitch, r + 2], [1, cs + 2]],
                            ),
                        )

                    # stage H: sort horizontal triples (col shifts)
                    pmin = wk.tile([128, r + 2, cs + 1], F32, tag="bufA")
                    pmax = wk.tile([128, r + 2, cs + 1], F32, tag="bufB")
                    eng("pmin").tensor_tensor(pmin[0:nj], t[0:nj, :, 0 : cs + 1], t[0:nj, :, 1 : cs + 2], MIN)
                    eng("pmax").tensor_tensor(pmax[0:nj], t[0:nj, :, 0 : cs + 1], t[0:nj, :, 1 : cs + 2], MAX)
                    rmin = wk.tile([128, r + 2, cs], F32, tag="bufC")
                    rmax = wk.tile([128, r + 2, cs], F32, tag="bufD")
                    e = wk.tile([128, r + 2, cs], F32, tag="bufE")
                    eng("rmin").tensor_tensor(rmin[0:nj], pmin[0:nj, :, 0:cs], pmin[0:nj, :, 1 : cs + 1], MIN)
                    eng("rmax").tensor_tensor(rmax[0:nj], pmax[0:nj, :, 0:cs], pmax[0:nj, :, 1 : cs + 1], MAX)
                    eng("e").tensor_tensor(e[0:nj], pmax[0:nj, :, 0:cs], t[0:nj, :, 2 : cs + 2], MIN)
                    rmed = wk.tile([128, r + 2, cs], F32, tag="bufF")
                    eng("rmed").tensor_tensor(rmed[0:nj], pmin[0:nj, :, 0:cs], e[0:nj], MAX)

                    # stage V: combine across rows (row shifts)
                    qmax = wk.tile([128, r + 1, cs], F32, tag="bufA")
                    eng("qmax").tensor_tensor(qmax[0:nj], rmin[0:nj, 0 : r + 1, :], rmin[0:nj, 1 : r + 2, :], MAX)
                    aa = wk.tile([128, r, cs], F32, tag="bufB")
                    eng("aa").tensor_tensor(aa[0:nj], qmax[0:nj, 0:r, :], qmax[0:nj, 1 : r + 1, :], MAX)
                    qmin = wk.tile([128, r + 1, cs], F32, tag="bufE")
                    eng("qmin").tensor_tensor(qmin[0:nj], rmax[0:nj, 0 : r + 1, :], rmax[0:nj, 1 : r + 2, :], MIN)
                    cc = wk.tile([128, r, cs], F32, tag="bufD")
                    eng("cc").tensor_tensor(cc[0:nj], qmin[0:nj, 0:r, :], qmin[0:nj, 1 : r + 1, :], MIN)
                    p2 = wk.tile([128, r + 1, cs], F32, tag="bufC")
                    eng("p2").tensor_tensor(p2[0:nj], rmed[0:nj, 0 : r + 1, :], rmed[0:nj, 1 : r + 2, :], MIN)
                    q2 = wk.tile([128, r + 1, cs], F32, tag="bufA")
                    eng("q2").tensor_tensor(q2[0:nj], rmed[0:nj, 0 : r + 1, :], rmed[0:nj, 1 : r + 2, :], MAX)
                    f2 = wk.tile([128, r, cs], F32, tag="bufE")
                    eng("f2").tensor_tensor(f2[0:nj], q2[0:nj, 0:r, :], rmed[0:nj, 2 : r + 2, :], MIN)
                    bb = wk.tile([128, r, cs], F32, tag="bufF")
                    eng("bb").tensor_tensor(bb[0:nj], p2[0:nj, 0:r, :], f2[0:nj], MAX)

                    # final med3(aa, bb, cc)
                    g = wk.tile([128, r, cs], F32, tag="bufC")
                    h2 = wk.tile([128, r, cs], F32, tag="bufA")
                    eng("g").tensor_tensor(g[0:nj], aa[0:nj], bb[0:nj], MIN)
                    eng("h2").tensor_tensor(h2[0:nj], aa[0:nj], bb[0:nj], MAX)
                    i2 = wk.tile([128, r, cs], F32, tag="bufE")
                    eng("i2").tensor_tensor(i2[0:nj], h2[0:nj], cc[0:nj], MIN)
                    med = io.tile([128, r, cs], F32, tag="med")
                    eng("med").tensor_tensor(med[0:nj], g[0:nj], i2[0:nj], MAX)

                    for k in range(nrb):
                        nc.sync.dma_start(
                            out=mk_ap(
                                out,
                                ch * h * w + (rb0 + k) * r * w,
                                [[cs, s_total], [w, r], [1, cs]],
                            ),
                            in_=med[k * s_total : (k + 1) * s_total],
                        )
                    rb0 += nrb
    nc.compile()
    return nc


_NC_CACHE = {}


def _get_nc():
    if "main" not in _NC_CACHE:
        _NC_CACHE["main"] = build_nc()
    return _NC_CACHE["main"]


def kernel(img: np.ndarray) -> np.ndarray:
    from concourse.bass_utils import run_bass_kernel_spmd

    img = np.ascontiguousarray(np.asarray(img, dtype=np.float32))
    assert img.shape == (B, C, H, W), img.shape
    nc = _get_nc()
    in_maps = [{"img": img[i]} for i in range(N_CORES)]
    res = run_bass_kernel_spmd(nc, in_maps, list(range(N_CORES)))
    return np.stack([res.results[i]["out"] for i in range(N_CORES)], axis=0)


# revision 16
# speedup vs baseline: 1.2329x; 1.2329x over previous
"""3x3 median filter (reflect padding) on Trainium2, data-parallel over 8 cores.

Problem: img [8, 3, 1024, 1024] f32 -> median-of-9 per pixel, same shape.
Sharding: batch dim across the 8 NeuronCores (one image per core, SPMD).

Per-core algorithm (image [3, 1024, 1024]):

  Pre-pass: build a reflect-padded copy of each channel in DRAM
  (pimg [C, H+2, PITCH]) by streaming rows through SBUF (the width
  padding costs two 1-column copies per row tile; the top/bottom
  padded rows are extra 1-row DMA writes).

  Main pass: every SBUF partition processes an independent sub-block
  of R=32 output rows x Cs=64 output cols, loading (R+2)x(Cs+2) with
  a 1-pixel halo from the padded image (one big overlapping-window
  DMA per 128-block iteration). Both window directions are then
  free-dimension shifts, so the whole median network runs as plain
  tensor_tensor min/max ops:

    stage H (col shifts): sort each horizontal triple -> rmin/rmed/rmax
    stage V (row shifts) + final:
        med9 = med3( max3_rows(rmin), med3_rows(rmed), min3_rows(rmax) )
    with med3(a,b,c) = max(min(a,b), min(max(a,b), c)).

  18 min/max tensor_tensor ops per pixel total, split between the
  Vector (DVE) and GPSIMD engines.
"""

import numpy as np

import concourse.bacc as bacc
import concourse.bass as bass
import concourse.mybir as mybir
from concourse.tile import TileContext

F32 = mybir.dt.float32
MIN = mybir.AluOpType.min
MAX = mybir.AluOpType.max

B, C, H, W = 8, 3, 1024, 1024
N_CORES = 8

# GPSIMD offload is rejected by this neuronxcc build ("Instruction engine
# check failed (Pool)" for TensorTensor/TensorScalarPtr), so the whole
# min/max network runs on the Vector engine.
GPS_DEFAULT = frozenset()


def mk_ap(handle, offset, dims):
    """Manual access pattern: dims = [[step, count], ...] in elements."""
    return bass.AP(handle, offset, [list(d) for d in dims])


def build_nc(c=C, h=H, w=W, r=32, cs=64, gpsimd_ops=GPS_DEFAULT):
    assert h % r == 0 and w % cs == 0
    rb_total = h // r          # row blocks per channel
    s_total = w // cs          # col strips per channel
    rb_per_iter = max(1, 128 // s_total)
    pitch = ((w + 2) + 7) // 8 * 8   # padded-image row pitch (f32 elems)
    ph = h + 2

    nc = bacc.Bacc(None, target_bir_lowering=False, debug=False)
    img = nc.dram_tensor("img", [c, h, w], F32, kind="ExternalInput")
    out = nc.dram_tensor("out", [c, h, w], F32, kind="ExternalOutput")
    # One padded-image tensor per channel so dependency tracking never
    # serializes channel ch's main pass against channel ch+1's pre-pass.
    pimgs = [nc.dram_tensor(f"pimg{i}", [ph, pitch], F32) for i in range(c)]

    def eng(name):
        return nc.gpsimd if name in gpsimd_ops else nc.vector

    with TileContext(nc) as tc:
        with (
            tc.tile_pool(name="pre", bufs=3) as pre,
            tc.tile_pool(name="io", bufs=3) as io,
            tc.tile_pool(name="wk", bufs=2) as wk,
        ):
            # ---------------- pre-pass: reflect-padded image in DRAM --------
            for ch in range(c):
                pimg = pimgs[ch]
                r0 = 0
                while r0 < h:
                    n = min(128, h - r0)
                    rt = pre.tile([128, w + 2], F32, tag="rt")
                    nc.sync.dma_start(out=rt[0:n, 1 : w + 1], in_=img[ch, r0 : r0 + n, :])
                    nc.scalar.copy(out=rt[0:n, 0:1], in_=rt[0:n, 2:3])
                    nc.scalar.copy(out=rt[0:n, w + 1 : w + 2], in_=rt[0:n, w - 1 : w])
                    nc.scalar.dma_start(
                        out=mk_ap(pimg, (1 + r0) * pitch, [[pitch, n], [1, w + 2]]),
                        in_=rt[0:n, :],
                    )
                    if r0 == 0:  # padded top row = img row 1 (reflect)
                        nc.scalar.dma_start(
                            out=mk_ap(pimg, 0, [[pitch, 1], [1, w + 2]]),
                            in_=rt[1:2, :],
                        )
                    if r0 + n == h:  # padded bottom row = img row h-2
                        pbot = h - 2 - r0
                        nc.scalar.dma_start(
                            out=mk_ap(pimg, (ph - 1) * pitch, [[pitch, 1], [1, w + 2]]),
                            in_=rt[pbot : pbot + 1, :],
                        )
                    r0 += n

            # ---------------- main pass: block-strip median -----------------
            # All compute works on the flattened [(r+2)*(cs+2)] per-partition
            # block; a column shift is +1, a row shift is +S where S = cs+2.
            # Positions that pair elements across a row wrap are garbage but
            # never feed a valid output pixel.
            S = cs + 2
            L = (r + 2) * S     # 2244: loaded block
            L1 = L - 1          # pmin/pmax
            L2 = L - 2          # rmin/rmax/e/rmed
            LQ = L2 - S         # qmax/qmin/p2/q2
            LO = LQ - S         # aa/cc/f2/bb/g/h2/i2/med
            for ch in range(c):
                pimg = pimgs[ch]
                rb0 = 0
                while rb0 < rb_total:
                    nrb = min(rb_per_iter, rb_total - rb0)
                    nj = nrb * s_total  # jobs (partitions) this iteration
                    t = io.tile([128, L], F32, tag="t")
                    t_view = t.rearrange("p (a b) -> p a b", b=S)
                    for k in range(nrb):
                        nc.sync.dma_start(
                            out=t_view[k * s_total : (k + 1) * s_total],
                            in_=mk_ap(
                                pimg,
                                (rb0 + k) * r * pitch,
                                [[cs, s_total], [pitch, r + 2], [1, cs + 2]],
                            ),
                        )

                    # stage H: sort horizontal triples (col shifts)
                    pmin = wk.tile([128, L1], F32, tag="bufA")
                    pmax = wk.tile([128, L1], F32, tag="bufB")
                    eng("pmin").tensor_tensor(pmin[0:nj], t[0:nj, 0:L1], t[0:nj, 1:L], MIN)
                    eng("pmax").tensor_tensor(pmax[0:nj], t[0:nj, 0:L1], t[0:nj, 1:L], MAX)
                    rmin = wk.tile([128, L2], F32, tag="bufC")
                    rmax = wk.tile([128, L2], F32, tag="bufD")
                    e = wk.tile([128, L2], F32, tag="bufE")
                    eng("rmin").tensor_tensor(rmin[0:nj], pmin[0:nj, 0:L2], pmin[0:nj, 1:L1], MIN)
                    eng("rmax").tensor_tensor(rmax[0:nj], pmax[0:nj, 0:L2], pmax[0:nj, 1:L1], MAX)
                    eng("e").tensor_tensor(e[0:nj], pmax[0:nj, 0:L2], t[0:nj, 2:L], MIN)
                    rmed = wk.tile([128, L2], F32, tag="bufF")
                    eng("rmed").tensor_tensor(rmed[0:nj], pmin[0:nj, 0:L2], e[0:nj], MAX)

                    # stage V: combine across rows (+S shifts)
                    qmax = wk.tile([128, LQ], F32, tag="bufA")
                    eng("qmax").tensor_tensor(qmax[0:nj], rmin[0:nj, 0:LQ], rmin[0:nj, S:L2], MAX)
                    aa = wk.tile([128, LO], F32, tag="bufB")
                    eng("aa").tensor_tensor(aa[0:nj], qmax[0:nj, 0:LO], qmax[0:nj, S:LQ], MAX)
                    qmin = wk.tile([128, LQ], F32, tag="bufE")
                    eng("qmin").tensor_tensor(qmin[0:nj], rmax[0:nj, 0:LQ], rmax[0:nj, S:L2], MIN)
                    cc = wk.tile([128, LO], F32, tag="bufD")
                    eng("cc").tensor_tensor(cc[0:nj], qmin[0:nj, 0:LO], qmin[0:nj, S:LQ], MIN)
                    p2 = wk.tile([128, LQ], F32, tag="bufC")
                    eng("p2").tensor_tensor(p2[0:nj], rmed[0:nj, 0:LQ], rmed[0:nj, S:L2], MIN)
                    q2 = wk.tile([128, LQ], F32, tag="bufA")
                    eng("q2").tensor_tensor(q2[0:nj], rmed[0:nj, 0:LQ], rmed[0:nj, S:L2], MAX)
                    f2 = wk.tile([128, LO], F32, tag="bufE")
                    eng("f2").tensor_tensor(f2[0:nj], q2[0:nj, 0:LO], rmed[0:nj, 2 * S : L2], MIN)
                    bb = wk.tile([128, LO], F32, tag="bufF")
                    eng("bb").tensor_tensor(bb[0:nj], p2[0:nj, 0:LO], f2[0:nj], MAX)

                    # final med3(aa, bb, cc)
                    g = wk.tile([128, LO], F32, tag="bufC")
                    h2 = wk.tile([128, LO], F32, tag="bufA")
                    eng("g").tensor_tensor(g[0:nj], aa[0:nj], bb[0:nj], MIN)
                    eng("h2").tensor_tensor(h2[0:nj], aa[0:nj], bb[0:nj], MAX)
                    i2 = wk.tile([128, LO], F32, tag="bufE")
                    eng("i2").tensor_tensor(i2[0:nj], h2[0:nj], cc[0:nj], MIN)
                    med = io.tile([128, L], F32, tag="med")
                    eng("med").tensor_tensor(med[0:nj, 0:LO], g[0:nj], i2[0:nj], MAX)

                    med_view = med.rearrange("p (a b) -> p a b", b=S)
                    for k in range(nrb):
                        nc.scalar.dma_start(
                            out=mk_ap(
                                out,
                                ch * h * w + (rb0 + k) * r * w,
                                [[cs, s_total], [w, r], [1, cs]],
                            ),
                            in_=med_view[k * s_total : (k + 1) * s_total, 0:r, 0:cs],
                        )
                    rb0 += nrb
    nc.compile()
    return nc


_NC_CACHE = {}


def _get_nc():
    if "main" not in _NC_CACHE:
        _NC_CACHE["main"] = build_nc()
    return _NC_CACHE["main"]


def kernel(img: np.ndarray) -> np.ndarray:
    from concourse.bass_utils import run_bass_kernel_spmd

    img = np.ascontiguousarray(np.asarray(img, dtype=np.float32))
    assert img.shape == (B, C, H, W), img.shape
    nc = _get_nc()
    in_maps = [{"img": img[i]} for i in range(N_CORES)]
    res = run_bass_kernel_spmd(nc, in_maps, list(range(N_CORES)))
    return np.stack([res.results[i]["out"] for i in range(N_CORES)], axis=0)
